# revision 3
# baseline (speedup 1.0000x reference)
"""LSTM ActionEncoder kernel for Trainium2 (8 NeuronCores, data-parallel on batch).

Reference computation (T=20, B=2048, IN=512, H=1024):
    x_emb = obs @ W_emb.T + b_emb                      # [T, B, 512]
    per step: gates = x_t @ W_ih.T + h @ W_hh.T + b    # LSTM cell, i,f,g,o
    returns h_final                                    # [B, 1024]

Device algorithm (per core, B=256):
  * The embedding + input projection fold on the host into a single
    [3 -> 4096] map applied to (obs_x, obs_y, 1):  gates_x = obs_aug @ Wfold.
  * Weight-moving matmul structure: stationary operand is an hT block, moving
    operand is W_hh (pre-transposed) in N=512 streams -> the PE runs at its
    ~216 ns/MM streaming rate with LDWEIGHTS hidden.  gates land as
    [batch-chunk, gate-col] PSUM tiles, fp32 accumulation.
  * The x-part fuses into each PSUM accumulation as 4-way-concurrent
    row-tiled (tile_position, K=32) matmuls.
  * Startup is HBM-arrival-bound (aggregate ~210-250 GB/s), so the early
    window ships a 4 MB fp8(e4m3) copy of W_hh used for steps 0-2 (adds
    ~2e-3 rel err, damped by the LSTM gates) while the 8 MB fp16 copy
    rides the scalar queue behind it for steps 3+.  obs/wfold content is
    4x-replicated across row groups, so only one 32-row copy ships and the
    replicas are SBUF->SBUF; c0 ships fp16 and converts on-chip.  Step 0
    consumes fp8 W k-tiles batch-interleaved (8 MMs = 1.73 us/tile) to stay
    above the arrival rate, and warm-up matmuls run during the first DMA so
    the HAM clock gate is at 8/8 when real work starts.  The sync queue
    carries only ht/fp8-W, then frees up for the per-step transposes.
  * Pointwise LSTM math: the four gate activations go PSUM->SBUF on ACT
    (banks free at ACT speed; DVE ops are pure-SBUF), products/sums on DVE,
    hidden under the PE's per-step matmul time.  At t=0 the v0 quadrants
    use o-first ACT order and the v1 chains are bank-sequenced (o,f,i,g)
    to match the PSUM bank free order.
  * h/obs/W are fp16 (PE upconverts to FP22); c stays fp32 in SBUF.
  * h_new is transposed for the next step via 128x128 XBAR DMA-transposes.
  * Last step's pointwise splits into column halves with early per-piece
    output DMAs to shorten the tail.

Layouts (per core, b-chunk bc in {0,1}, gate g in {i,f,g,o}, v in {0,1} =
which 512-wide half of the 1024 h-columns, k = 128-row h chunk):
  wv_packed/wv8_packed [16,128,2048]:
      [v*8+k, p, 512*g+c] = W_hh[1024*g+512*v+c, 128*k+p]
  wfold  [32,4096]: rows {0,1,2} = (W_ih@W_emb).T and the folded bias
  obs_aug [32, T*256]: [0, 256t+c]=obs_x, [1,...]=obs_y, [2,...]=1.0
  h0t    [128,2048]: [p, 256*k+128*bc+w] = h0[128*bc+w, 128*k+p]
  c0b/h_out [2,128,1024]: [bc, p, hh] = state[128*bc+p, hh]
"""

import numpy as np

T = 20
H = 1024
NCORES = 8
B = 256  # batch per core
FP8_STEPS = 3  # steps 0..FP8_STEPS-1 use the fp8 W copy

_CACHED_NC = None
LAST_RESULT = None  # BassKernelResults of the most recent run (for test harness)


def _build_program():
    import concourse.mybir as mybir
    import concourse.tile as tile
    from concourse import bacc

    f32 = mybir.dt.float32
    f16 = mybir.dt.float16
    f8 = mybir.dt.float8e4
    AFT = mybir.ActivationFunctionType

    nc = bacc.Bacc("TRN2", target_bir_lowering=False)
    wv8_dram = nc.dram_tensor("wv8_packed", (16, 128, 2048), f8, kind="ExternalInput")
    wv_dram = nc.dram_tensor("wv_packed", (16, 128, 2048), f16, kind="ExternalInput")
    wfold_dram = nc.dram_tensor("wfold", (32, 4096), f16, kind="ExternalInput")
    obs_dram = nc.dram_tensor("obs_aug", (32, T * B), f16, kind="ExternalInput")
    h0_dram = nc.dram_tensor("h0t", (128, 2048), f16, kind="ExternalInput")
    c0_dram = nc.dram_tensor("c0b", (2, 128, 1024), f16, kind="ExternalInput")
    out_dram = nc.dram_tensor("h_out", (2, 128, 1024), f16, kind="ExternalOutput")

    with tile.TileContext(nc) as tc:
        with (
            tc.tile_pool(name="wpool", bufs=1) as wpool,
            tc.tile_pool(name="spool", bufs=1) as spool,
            tc.tile_pool(name="gpool", bufs=2) as gpool,
            tc.tile_pool(name="ppool", bufs=2, space="PSUM") as ppool,
        ):
            obs_sb = wpool.tile([128, T * B], f16, name="obs_sb")
            wfold_sb = wpool.tile([128, 4096], f16, name="wfold_sb")
            ht = [spool.tile([128, 2048], f16, name=f"ht{s}") for s in range(2)]
            cs = [spool.tile([128, 1024], f32, name=f"cs{bc}") for bc in range(2)]
            c16 = spool.tile([128, 2048], f16, name="c16")
            wv8 = {
                (v, k): wpool.tile([128, 2048], f8, name=f"wv8_{v}_{k}")
                for v in range(2)
                for k in range(8)
            }
            wv16 = {
                (v, k): wpool.tile([128, 2048], f16, name=f"wv_{v}_{k}")
                for v in range(2)
                for k in range(8)
            }

            # sync queue: ht_lo then the fp8 W evens; frees up for transposes.
            nc.sync.dma_start(ht[0][:, 0:1024], h0_dram[:, 0:1024])  # k=0..3
            for v in range(2):
                for k in range(0, 8, 2):
                    nc.sync.dma_start(wv8[(v, k)][:], wv8_dram[8 * v + k])
            # scalar queue: fp8 W odds, then the full fp16 W for steps 3+.
            for v in range(2):
                for k in range(1, 8, 2):
                    nc.scalar.dma_start(wv8[(v, k)][:], wv8_dram[8 * v + k])
            for v in range(2):
                for k in range(8):
                    nc.scalar.dma_start(wv16[(v, k)][:], wv_dram[8 * v + k])

            # gpsimd queue: small/latency-critical pieces in need-order.
            TB2 = 2 * B
            nc.gpsimd.dma_start(obs_sb[0:32, 0:TB2], obs_dram[:, 0:TB2])  # t=0,1
            nc.gpsimd.dma_start(ht[0][:, 1024:2048], h0_dram[:, 1024:2048])
            nc.gpsimd.dma_start(obs_sb[32:64, 0:TB2], obs_sb[0:32, 0:TB2])
            nc.gpsimd.dma_start(obs_sb[64:128, 0:TB2], obs_sb[0:64, 0:TB2])
            nc.gpsimd.dma_start(wfold_sb[0:32, :], wfold_dram[:])
            nc.gpsimd.dma_start(wfold_sb[32:64, :], wfold_sb[0:32, :])
            nc.gpsimd.dma_start(wfold_sb[64:128, :], wfold_sb[0:64, :])
            for bc in range(2):
                nc.gpsimd.dma_start(c16[:, 1024 * bc : 1024 * (bc + 1)], c0_dram[bc])
            for bc in range(2):
                nc.vector.tensor_copy(cs[bc][:], c16[:, 1024 * bc : 1024 * (bc + 1)])
            nc.gpsimd.dma_start(obs_sb[0:32, TB2:], obs_dram[:, TB2:])  # t=2..19
            nc.gpsimd.dma_start(obs_sb[32:64, TB2:], obs_sb[0:32, TB2:])
            nc.gpsimd.dma_start(obs_sb[64:128, TB2:], obs_sb[0:64, TB2:])

            def xmm(ps, bc, v, t, g, start, stop):
                # x-part: row-tiled K=32 matmul (concurrent across row groups).
                nc.tensor.matmul(
                    ps[:],
                    obs_sb[32 * g : 32 * g + 32, B * t + 128 * bc : B * t + 128 * bc + 128],
                    wfold_sb[
                        32 * g : 32 * g + 32,
                        1024 * g + 512 * v : 1024 * g + 512 * v + 512,
                    ],
                    start=start,
                    stop=stop,
                    tile_position=(32 * g, 0),
                )

            def wmm(ps, bc, v, k, g, t, start, stop):
                wt = wv8 if t < FP8_STEPS else wv16
                nc.tensor.matmul(
                    ps[:],
                    ht[t % 2][:, 256 * k + 128 * bc : 256 * k + 128 * bc + 128],
                    wt[(v, k)][:, 512 * g : 512 * (g + 1)],
                    start=start,
                    stop=stop,
                )

            def pointwise(ps, bc, v, t, hnew, o_first=False, split=1):
                # gates -> (h_new, c_new).  All four activations go PSUM->SBUF
                # so banks free at ACT speed and DVE never reads PSUM.
                pi, pf, pg, po = ps
                abo = gpool.tile([128, 2048], f32, name="abo", tag="abo")
                W = 512 // split
                for s in range(split):
                    c0_, c1_ = s * W, (s + 1) * W
                    o = abo[:, 0 + c0_ : 0 + c1_]
                    a = abo[:, 512 + c0_ : 512 + c1_]
                    b = abo[:, 1024 + c0_ : 1024 + c1_]
                    f = abo[:, 1536 + c0_ : 1536 + c1_]
                    csl = cs[bc][:, 512 * v + c0_ : 512 * v + c1_]
                    if o_first:
                        nc.scalar.activation(o, po[:, c0_:c1_], AFT.Sigmoid)
                        nc.scalar.activation(f, pf[:, c0_:c1_], AFT.Sigmoid)
                        nc.scalar.activation(a, pi[:, c0_:c1_], AFT.Sigmoid)
                        nc.scalar.activation(b, pg[:, c0_:c1_], AFT.Tanh)
                    else:
                        nc.scalar.activation(a, pi[:, c0_:c1_], AFT.Sigmoid)
                        nc.scalar.activation(b, pg[:, c0_:c1_], AFT.Tanh)
                        nc.scalar.activation(f, pf[:, c0_:c1_], AFT.Sigmoid)
                        nc.scalar.activation(o, po[:, c0_:c1_], AFT.Sigmoid)
                    nc.vector.tensor_mul(f, f, csl)  # f*c
                    nc.vector.tensor_mul(a, a, b)  # i*g
                    nc.vector.tensor_add(csl, f, a)  # c_new
                    nc.scalar.activation(b, csl, AFT.Tanh)
                    nc.vector.tensor_mul(
                        hnew[bc][:, 512 * v + c0_ : 512 * v + c1_], o, b
                    )  # h_new (fp16)
                    if t == T - 1:
                        nc.gpsimd.dma_start(
                            out_dram[bc][:, 512 * v + c0_ : 512 * v + c1_],
                            hnew[bc][:, 512 * v + c0_ : 512 * v + c1_],
                        )

            def transposes(bc, v, t, hnew):
                # hT blocks for the next step via 128x128 XBAR DMA transposes.
                wr = (t + 1) % 2
                for k in range(4 * v, 4 * v + 4):
                    nc.sync.dma_start(
                        ht[wr][:, 256 * k + 128 * bc : 256 * k + 128 * bc + 128],
                        hnew[bc][:, 128 * k : 128 * (k + 1)],
                        transpose=True,
                    )

            # ---- warm-up: keep the PE busy during the initial DMA window so
            # the HAM clock gate reaches 8/8 before real matmuls start.
            ps_warm = ppool.tile([128, 512], f32, name="ps0", tag="ps0")
            for _ in range(24):
                nc.tensor.matmul(
                    ps_warm[0:64, 0:64],
                    ht[0][:, 0:64],
                    ht[0][:, 64:128],
                    start=True,
                    stop=True,
                )

            # ---- t = 0: arrival-bound; consume fp8 W k-tiles batch-interleaved
            # so per-tile demand (8 MMs = 1.73 us) stays above the HBM arrival
            # rate.  v1 chains are bank-sequenced to match the order the v0
            # pointwise frees PSUM banks.
            hnew0 = [
                gpool.tile([128, 1024], f16, name=f"hnew{bc}", tag=f"hnew{bc}")
                for bc in range(2)
            ]
            ps0 = {
                bc: [
                    ppool.tile([128, 512], f32, name=f"ps{g}", tag=f"ps{g}")
                    for g in range(4)
                ]
                for bc in range(2)
            }
            for k in range(8):
                for bc in range(2):
                    for g in range(4):
                        wmm(ps0[bc][g], bc, 0, k, g, 0, start=(k == 0), stop=(k == 7))
                if k == 5:
                    # x-packs slot into arrival slack mid-chain.
                    for bc in range(2):
                        for g in range(4):
                            xmm(ps0[bc][g], bc, 0, 0, g, start=False, stop=False)
            for bc in range(2):
                pointwise(ps0[bc], bc, 0, 0, hnew0, o_first=True)
                transposes(bc, 0, 0, hnew0)
            for bc in range(2):
                ps1 = [
                    ppool.tile([128, 512], f32, name=f"ps{g}", tag=f"ps{g}")
                    for g in range(4)
                ]
                for k in range(8):
                    for g in (3, 1, 0, 2):  # PSUM bank free order (o,f,i,g)
                        wmm(ps1[g], bc, 1, k, g, 0, start=(k == 0), stop=False)
                for g in range(4):
                    xmm(ps1[g], bc, 1, 0, g, start=False, stop=True)
                pointwise(ps1, bc, 1, 0, hnew0)
                transposes(bc, 1, 0, hnew0)

            # ---- steady state t = 1..T-1 (fp8 W until FP8_STEPS)
            for t in range(1, T):
                hnew = [
                    gpool.tile([128, 1024], f16, name=f"hnew{bc}", tag=f"hnew{bc}")
                    for bc in range(2)
                ]
                for bc in range(2):
                    for v in range(2):
                        ps = [
                            ppool.tile([128, 512], f32, name=f"ps{g}", tag=f"ps{g}")
                            for g in range(4)
                        ]
                        # x-part first: h-independent, so the PE has ready work
                        # at the step boundary; start=True opens each bank.
                        for g in range(4):
                            xmm(ps[g], bc, v, t, g, start=True, stop=False)
                        for g in range(4):
                            for k in range(8):
                                wmm(ps[g], bc, v, k, g, t, start=False, stop=(k == 7))
                        pointwise(ps, bc, v, t, hnew, split=(2 if t == T - 1 else 1))
                        if t < T - 1:
                            transposes(bc, v, t, hnew)

    nc.compile()
    return nc


def _host_prep(inputs):
    import ml_dtypes

    obs = np.asarray(inputs["obs_traj"], dtype=np.float32)
    h0 = np.asarray(inputs["h0"], dtype=np.float32)
    c0 = np.asarray(inputs["c0"], dtype=np.float32)
    W_emb = np.asarray(inputs["W_emb"], dtype=np.float32)
    b_emb = np.asarray(inputs["b_emb"], dtype=np.float32)
    W_ih = np.asarray(inputs["W_ih"], dtype=np.float32)
    W_hh = np.asarray(inputs["W_hh"], dtype=np.float32)
    b_ih = np.asarray(inputs["b_ih"], dtype=np.float32)
    b_hh = np.asarray(inputs["b_hh"], dtype=np.float32)

    Wf = (W_ih @ W_emb).astype(np.float32)  # [4096, 2]
    biasf = (W_ih @ b_emb + b_ih + b_hh).astype(np.float32)
    wfold = np.zeros((32, 4096), np.float16)
    wfold[0] = Wf[:, 0]
    wfold[1] = Wf[:, 1]
    wfold[2] = biasf

    # [v*8+k, p, 512*g+c] = W_hh[1024*g+512*v+c, 128*k+p]
    wv_pack_f32 = np.ascontiguousarray(
        W_hh.reshape(4, 2, 512, 8, 128).transpose(1, 3, 4, 0, 2).reshape(16, 128, 2048)
    )
    wv_packed = wv_pack_f32.astype(np.float16)
    wv8_packed = wv_pack_f32.astype(ml_dtypes.float8_e4m3)

    in_maps = []
    for c in range(NCORES):
        sl = slice(B * c, B * (c + 1))
        # obs_aug [32, T*B]: rows {0,1,2} = obs_x, obs_y, 1.0
        obs_aug = np.zeros((32, T * B), np.float16)
        ob = obs[:, sl, :]  # [T, B, 2]
        obs_aug[0] = ob[:, :, 0].reshape(T * B)
        obs_aug[1] = ob[:, :, 1].reshape(T * B)
        obs_aug[2] = 1.0
        # h0t[p, 256k+128bc+w] = h0[128bc+w, 128k+p]
        h0t = np.ascontiguousarray(
            h0[sl].reshape(2, 128, 8, 128).transpose(3, 2, 0, 1).reshape(128, 2048)
        ).astype(np.float16)
        c0b = np.ascontiguousarray(c0[sl].reshape(2, 128, 1024)).astype(np.float16)
        in_maps.append(
            dict(
                wv_packed=wv_packed,
                wv8_packed=wv8_packed,
                wfold=wfold,
                obs_aug=obs_aug,
                h0t=h0t,
                c0b=c0b,
            )
        )
    return in_maps


def _unpack_out(o):  # [2, 128, 1024] -> [256, 1024]
    return o.reshape(B, H)


def kernel(**inputs) -> np.ndarray:
    global _CACHED_NC, LAST_RESULT
    from concourse.bass_utils import run_bass_kernel_spmd

    in_maps = _host_prep(inputs)
    if _CACHED_NC is None:
        _CACHED_NC = _build_program()
    res = run_bass_kernel_spmd(_CACHED_NC, in_maps, core_ids=list(range(NCORES)))
    LAST_RESULT = res
    out = np.concatenate(
        [_unpack_out(res.results[c]["h_out"]) for c in range(NCORES)], axis=0
    )
    return np.ascontiguousarray(out.astype(np.float32))
